# revision 17
# baseline (speedup 1.0000x reference)
"""Bass/Trainium2 kernel for batched multi-head self-attention.

Module math (per batch b):
    q = vec @ Wq; k = vec @ Wk; v = vec @ Wv            (per head h, dim d=16)
    S = q k^T / sqrt(d);  P = softmax_j(S);  recv = P v
    out = recv @ Wo

Sharding: data-parallel over batch (8 batches -> 8 NeuronCores), weights
replicated. Each core runs an identical Bass program on its vec slice.

Per-head pipeline on a core:
  1. form1: S[i, j] via K=64 zero-padded f32r matmuls; DVE row-max (negated).
  2. "m-dance": the per-row -max vector is transposed (PE) and DMA-flattened
     into an augmentation partition of the f32r Q^T tensor.
  3. S'^T[j, i] = KT-aug.T @ QT-aug (K=64 zero-padded f32r): the aug row
     (ones x -max) subtracts the row max inside the matmul, so ACT exp with
     scale=1/4 needs no per-column bias. exp -> P^T in fp16.  (The f32r
     rounding of the aug term only shifts the shared max, which cancels in
     the softmax ratio.)
  4. PV: lhsT = [V_h | 1] fp16 (M=17) accumulates recv^T plus the softmax
     denominator in one stream; both heads + both column halves of a round
     share one PSUM tile (disjoint partition strips), so one DVE copy drains
     the round.
  5. Tail: reciprocal + PE expand-matmul + fused normalize, Wo projection.

All big matmuls run at 1 cycle/column: f32r for QK/projections (fp32 data,
TF32-like internal precision), fp16 for PV. K=64 zero-padding keeps the PE
HAM activity monitor warm (2.4 GHz); K=16 matmuls read as ~13% array
activity and leave the PE throttled at 1.2 GHz.

Shapes (hardcoded): vec [8, 1024, 128]; Wq/Wk/Wv [128, 8, 16]; Wo [8, 16, 128].
"""

import sys

sys.path.insert(0, "/opt/trn_rl_repo")

from contextlib import ExitStack

import numpy as np

import concourse.bacc as bacc
import concourse.tile as tile
from concourse import mybir
from concourse.bass_utils import run_bass_kernel_spmd
from concourse.masks import make_identity

F32 = mybir.dt.float32
F32R = mybir.dt.float32r
F16 = mybir.dt.float16
BF16 = mybir.dt.bfloat16
Exp = mybir.ActivationFunctionType.Exp

B, N, X, H, D = 8, 1024, 128, 8, 16
NCHUNK = N // 128          # 8 chunks of 128 along the token dim
SCALE = 0.125              # 1/sqrt(16), halved: S' = 2s - 2m
NR = 4                     # qk rounds: 2 heads each at strips {0, 64}

_CACHED_NC = None


def build_nc():
    """Build the per-core Bass program (identical on all cores)."""
    nc = bacc.Bacc("TRN2")

    # DRAM I/O. Weight tensors arrive pre-permuted from numpy (see kernel()).
    d_wq = [nc.dram_tensor(f"wq{r}", (X, 128), F32, kind="ExternalInput")
            for r in range(NR)]
    d_wk = [nc.dram_tensor(f"wk{r}", (X, 128), F32, kind="ExternalInput")
            for r in range(NR)]
    d_wv = nc.dram_tensor("wv", (X, 128), F32, kind="ExternalInput")
    d_wo = nc.dram_tensor("wo", (128, X), F32, kind="ExternalInput")
    d_vec = nc.dram_tensor("vec", (N, X), F32, kind="ExternalInput")
    d_e8 = nc.dram_tensor("e8c", (H, 128), F32, kind="ExternalInput")
    d_ones = nc.dram_tensor("ones", (1, N), F32, kind="ExternalInput")
    d_out = nc.dram_tensor("out", (N, X), F32, kind="ExternalOutput")

    with tile.TileContext(nc) as tc, ExitStack() as top:
        const = top.enter_context(tc.tile_pool(name="const", bufs=1))
        ident = const.tile([128, 128], F32)
        make_identity(nc, ident)

        # Projections run in full fp32 (accuracy: the QK rounding error of
        # f32r is amplified through exp). QT/KT/ones/m8 are F32R *tiles*
        # (full fp32 bits in SBUF; the TF32-style rounding happens at the PE
        # input) so the S passes stream at 1 cycle/col. f32r tiles must be
        # filled by engine casts or same-dtype DMAs; a bitcast-DMA fill
        # crashes the compiler once a matmul consumes them.
        w_sb = {}
        for name, dram in ([(f"wq{r}", d_wq[r]) for r in range(NR)]
                           + [(f"wk{r}", d_wk[r]) for r in range(NR)]
                           + [("wv", d_wv), ("wo", d_wo)]):
            t = const.tile([128, 128], F32, tag=f"w_{name}", name=f"w_{name}")
            nc.sync.dma_start(out=t[:], in_=dram[:, :])
            w_sb[name] = t
        wo_sb = w_sb["wo"]
        ones_f = const.tile([1, N], F32, tag="ones_f")
        nc.sync.dma_start(out=ones_f[:], in_=d_ones[:, :])
        ones_r = const.tile([1, N], F32R, tag="ones_r")
        nc.scalar.copy(ones_r[:], ones_f[:])   # 0.5-valued (see QT/KT note)

        vecT = const.tile([128, N], F32, tag="vecT")       # [x, n]
        # QT/KT per head, double-packed to K=128: rows [0,16) and [64,80)
        # both hold the head's 16 dims (so every S matmul drives the full
        # 128-partition array -- K=64 f32r reads as 50% HAM activity and
        # the PE gets clock-throttled to 1.2 GHz). Row 16 and 80 are aug
        # rows: KT holds 0.5 in both (so S' = 2s + aug once), QT holds
        # -2*rowmax after the m-dance; remaining rows are zero. The exp
        # scale halves to 0.125 to undo the doubling.
        QT = {h: const.tile([128, N], F32R, tag=f"qt{h}", name=f"qt{h}")
              for h in range(H)}
        KT = {h: const.tile([128, N], F32R, tag=f"kt{h}", name=f"kt{h}")
              for h in range(H)}
        # V layout: [128 j-in-chunk, jc, 17*h + d], col 17h+16 = ones.
        V_sb = const.tile([128, NCHUNK, 17 * H], F16, tag="vsb")
        pt_pool = top.enter_context(tc.tile_pool(name="pt", bufs=4))
        raw_pool = top.enter_context(tc.tile_pool(name="raw", bufs=2))
        recvT = const.tile([128, N], F32, tag="recvT")     # [(h d), i]
        recvN = const.tile([128, N], F32, tag="recvN")     # normalized
        rden = const.tile([H, N], F32, tag="rden")
        e8 = const.tile([H, 128], F32, tag="e8")           # expand matrix
        mha_sb = const.tile([128, NCHUNK, X], F32, tag="mha")

        nc.sync.dma_start(out=e8[:], in_=d_e8[:, :])
        v_heads = V_sb[:].rearrange("p c (h s) -> p c h s", h=H)
        nc.vector.memset(v_heads[:, :, :, 16:17], 1.0)

        # ---- Phase 0: vecT via PE transposes; projections. ----
        with tc.tile_pool(name="stage", bufs=3) as stage, \
                tc.tile_pool(name="ps0", bufs=2, space="PSUM") as ps0, \
                tc.tile_pool(name="ps0b", bufs=2, space="PSUM") as ps0b:
            for c in range(NCHUNK):
                vt = stage.tile([128, 128], F32, tag="vstage")
                nc.sync.dma_start(out=vt[:], in_=d_vec[c * 128:(c + 1) * 128, :])
                pt_ = ps0b.tile([128, 128], F32, tag="trp")
                nc.tensor.transpose(pt_[:, :], vt[:], ident[:])
                nc.scalar.copy(vecT[:, c * 128:(c + 1) * 128], pt_[:, :])

            # QT/KT projections: psum = W.T @ vecT  -> [hd-pos, n];
            # strip t=h%2 of the pair lands at partitions [64t, 64t+17),
            # the rest of that 64-row half is zero (strip-packed W). Copy
            # the full half (strip + zeros) and DMA-duplicate it into the
            # other half: the tile is fully initialized with no memsets.
            for rnd in range(NR):
                for wname, dsts in ((f"wq{rnd}", QT), (f"wk{rnd}", KT)):
                    p = ps0.tile([128, N], F32, tag="proj")
                    for half in range(2):
                        sl = slice(half * 512, (half + 1) * 512)
                        nc.tensor.matmul(p[:, sl], w_sb[wname][:],
                                         vecT[:, sl], start=True, stop=True)
                    for t, h in enumerate((2 * rnd, 2 * rnd + 1)):
                        s0, s1 = 64 * t, 64 * (1 - t)
                        nc.scalar.copy(dsts[h][s0:s0 + 64, :],
                                       p[s0:s0 + 64, :])
                        nc.sync.dma_start(out=dsts[h][s1:s1 + 64, :],
                                          in_=dsts[h][s0:s0 + 64, :])
            # 0.5 rows of KT aug partitions (f32r -> f32r DMA)
            for h in range(H):
                for t in range(2):
                    nc.sync.dma_start(
                        out=KT[h][64 * t + 16:64 * t + 17, :],
                        in_=ones_r[:])

            # V projection: per chunk [j, hd] = vecT[:,chunk].T @ Wv
            for c in range(NCHUNK):
                pv = ps0.tile([128, 128], F32, tag="projv")
                nc.tensor.matmul(pv[:, :], vecT[:, c * 128:(c + 1) * 128],
                                 w_sb["wv"][:], start=True, stop=True)
                dst = V_sb[:, c, :].rearrange("p (h s) -> p h s", h=H)
                src = pv[:, :].rearrange("p (h d) -> p h d", h=H)
                nc.scalar.copy(dst[:, :, 0:16], src[:])

        # ---- Main loop over heads. ----
        with tc.tile_pool(name="small", bufs=6) as small, \
                tc.tile_pool(name="psm", bufs=3, space="PSUM") as psm, \
                tc.tile_pool(name="ppv", bufs=1, space="PSUM") as ppv:
            def emit_form1(rnd, c, m_hs):
                """One i-chunk of the f32r max-pass for both heads of rnd."""
                f1s = {}
                for h in (2 * rnd, 2 * rnd + 1):
                    f1 = psm.tile([128, N], F32, tag="big",
                                  name=f"f1_{h}_{c}")
                    f1s[h] = f1
                    for half in range(2):
                        sl = slice(half * 512, (half + 1) * 512)
                        nc.tensor.matmul(
                            f1[:, sl],
                            QT[h][:, c * 128:(c + 1) * 128],
                            KT[h][:, sl], start=True, stop=True)
                for h in (2 * rnd, 2 * rnd + 1):
                    nc.vector.tensor_reduce(
                        m_hs[h][:, c:c + 1], f1s[h][:, :],
                        axis=mybir.AxisListType.X,
                        op=mybir.AluOpType.max, negate=True)

            def new_mhs(rnd):
                return {h: small.tile([128, NCHUNK], F32, tag="mh",
                                      name=f"mh{h}")
                        for h in (2 * rnd, 2 * rnd + 1)}

            # prologue: round 0 max-pass
            m_cur = new_mhs(0)
            for c in range(NCHUNK):
                emit_form1(0, c, m_cur)

            for rnd in range(NR):
                pair = (2 * rnd, 2 * rnd + 1)

                # m-dance per head: -2*rowmax -> both aug rows of QT[h].
                for h in pair:
                    trp = psm.tile([128, N], F32, tag="big",
                                   name=f"trp{h}")
                    nc.tensor.transpose(trp[0:NCHUNK, 0:128],
                                        m_cur[h][:], ident[:])
                    m8 = small.tile([NCHUNK, 128], F32R, tag="m8",
                                    name=f"m8_{h}")
                    nc.scalar.copy(m8[:], trp[0:NCHUNK, 0:128])
                    nc.sync.dma_start(out=QT[h][16:17, :], in_=m8[:])
                    nc.sync.dma_start(out=QT[h][80:81, :], in_=m8[:])

                # S'^T + exp, strip-interleaved across the head pair,
                # with next round's max-pass chunks woven in.
                m_nxt = new_mhs(rnd + 1) if rnd + 1 < NR else None
                PTs = {h: pt_pool.tile([128, NCHUNK * N], F16, tag="pt",
                                       name=f"pt{h}")
                       for h in pair}
                prv = ppv.tile([128, N], F32, tag="pv", name=f"prv{rnd}")
                def emit_pv(jc):
                    # PV chunk jc, both heads and halves, into one PSUM
                    # tile (partition strips 0:17 / 32:49, col halves).
                    # Lagged one jc behind the ST/exp pipeline so exp(jc)
                    # is already done when the PE reaches it (the in-order
                    # PE queue head-of-line blocks on unmet waits), and the
                    # low-activity M=17 streams average with the K=128 S
                    # matmuls inside the HAM throttle window.
                    for h in pair:
                        po = 32 * (h % 2)
                        for half in range(2):
                            nc.tensor.matmul(
                                prv[po:po + 17,
                                    half * 512:(half + 1) * 512],
                                V_sb[:, jc, 17 * h:17 * h + 17],
                                PTs[h][:, jc * N + half * 512:
                                        jc * N + (half + 1) * 512],
                                start=(jc == 0), stop=(jc == NCHUNK - 1))

                for jc in range(NCHUNK):
                    if m_nxt is not None:
                        emit_form1(rnd + 1, jc, m_nxt)
                    sts = {}
                    for h in pair:
                        st = psm.tile([128, N], F32, tag="big",
                                      name=f"st_{h}_{jc}")
                        sts[h] = st
                        for half in range(2):
                            sl = slice(half * 512, (half + 1) * 512)
                            nc.tensor.matmul(
                                st[:, sl],
                                KT[h][:, jc * 128:(jc + 1) * 128],
                                QT[h][:, sl], start=True, stop=True)
                    for h in pair:
                        nc.scalar.activation(
                            PTs[h][:, jc * N:jc * N + N], sts[h][:, :],
                            Exp, bias=0.0, scale=SCALE)
                    if jc > 0:
                        emit_pv(jc - 1)
                emit_pv(NCHUNK - 1)
                rawt = raw_pool.tile([64, N], F32, tag="raw",
                                     name=f"raw{rnd}")
                nc.vector.tensor_copy(rawt[0:49, :], prv[0:49, :])
                den2 = raw_pool.tile([2, N], F32, tag="den2",
                                     name=f"den2_{rnd}")
                for t, h in enumerate(pair):
                    po = 32 * (h % 2)
                    nc.sync.dma_start(out=recvT[16 * h:16 * h + 16, :],
                                      in_=rawt[po:po + 16, :])
                    nc.sync.dma_start(out=den2[t:t + 1, :],
                                      in_=rawt[po + 16:po + 17, :])
                # per-round reciprocal on standalone 2-row tiles (the DVE
                # RECIPROCAL op miscompiles on partition sub-slices), DMA'd
                # into the rden rows the tail expand matmul reads.
                rd2 = raw_pool.tile([2, N], F32, tag="rd2",
                                    name=f"rd2_{rnd}")
                nc.vector.reciprocal(rd2[:], den2[:])
                nc.sync.dma_start(out=rden[pair[0]:pair[0] + 2, :],
                                  in_=rd2[:])
                m_cur = m_nxt

        # ---- Tail: normalize + output projection. ----
        with tc.tile_pool(name="pst", bufs=2, space="PSUM") as pst, \
                tc.tile_pool(name="pstb", bufs=2, space="PSUM") as pstb:
            pe_ = pst.tile([128, N], F32, tag="expand")
            for half in range(2):
                sl = slice(half * 512, (half + 1) * 512)
                nc.tensor.matmul(pe_[:, sl], e8[:], rden[:, sl],
                                 start=True, stop=True)
            nc.vector.tensor_mul(recvN[:], recvT[:], pe_[:, :])
            for c in range(NCHUNK):
                po = pstb.tile([128, 128], F32, tag="mha")
                nc.tensor.matmul(po[:, :], recvN[:, c * 128:(c + 1) * 128],
                                 wo_sb[:], start=True, stop=True)
                nc.scalar.copy(mha_sb[:, c, :], po[:, :])
                nc.sync.dma_start(out=d_out[c * 128:(c + 1) * 128, :],
                                  in_=mha_sb[:, c, :])

    nc.finalize()
    return nc


def _permute_weights(Wq, Wk, Wv, Wo):
    """Numpy-side weight layout prep: strip-pack with K=64 zero padding."""
    def strip_pack(W, heads):
        out = np.zeros((X, 128), dtype=np.float32)
        for t, h in enumerate(heads):
            out[:, 64 * t:64 * t + 16] = W[:, h, :]
        return out

    e8c = np.zeros((H, 128), dtype=np.float32)
    for h in range(H):
        e8c[h, 16 * h:16 * h + 16] = 1.0
    d = dict(
        wv=np.ascontiguousarray(Wv.reshape(X, 128)),
        wo=np.ascontiguousarray(Wo.reshape(128, X)),
        e8c=e8c, ones=np.full((1, N), 0.5, dtype=np.float32),
    )
    for r in range(NR):
        d[f"wq{r}"] = strip_pack(Wq, [2 * r, 2 * r + 1])
        d[f"wk{r}"] = strip_pack(Wk, [2 * r, 2 * r + 1])
    return d


def kernel(Wq, Wk, Wv, Wo, vec, trace=False, tmpdir=None):
    global _CACHED_NC
    if _CACHED_NC is None:
        _CACHED_NC = build_nc()
    nc = _CACHED_NC

    w = _permute_weights(np.asarray(Wq, np.float32), np.asarray(Wk, np.float32),
                         np.asarray(Wv, np.float32), np.asarray(Wo, np.float32))
    vec = np.asarray(vec, np.float32)
    in_maps = [dict(w, vec=np.ascontiguousarray(vec[b])) for b in range(B)]
    res = run_bass_kernel_spmd(nc, in_maps, core_ids=list(range(B)),
                               trace=trace, tmpdir=tmpdir)
    out = np.stack([res.results[b]["out"] for b in range(B)])
    if trace:
        return out, res
    return out


# revision 18
# speedup vs baseline: 1.0873x; 1.0873x over previous
"""Bass/Trainium2 kernel for batched multi-head self-attention.

Module math (per batch b):
    q = vec @ Wq; k = vec @ Wk; v = vec @ Wv            (per head h, dim d=16)
    S = q k^T / sqrt(d);  P = softmax_j(S);  recv = P v
    out = recv @ Wo

Sharding: data-parallel over batch (8 batches -> 8 NeuronCores), weights
replicated. Each core runs an identical Bass program on its vec slice.

Per-head pipeline on a core:
  1. form1: S[i, j] via K=64 zero-padded f32r matmuls; DVE row-max (negated).
  2. "m-dance": the per-row -max vector is transposed (PE) and DMA-flattened
     into an augmentation partition of the f32r Q^T tensor.
  3. S'^T[j, i] = KT-aug.T @ QT-aug (K=64 zero-padded f32r): the aug row
     (ones x -max) subtracts the row max inside the matmul, so ACT exp with
     scale=1/4 needs no per-column bias. exp -> P^T in fp16.  (The f32r
     rounding of the aug term only shifts the shared max, which cancels in
     the softmax ratio.)
  4. PV: lhsT = [V_h | 1] fp16 (M=17) accumulates recv^T plus the softmax
     denominator in one stream; both heads + both column halves of a round
     share one PSUM tile (disjoint partition strips), so one DVE copy drains
     the round.
  5. Tail: reciprocal + PE expand-matmul + fused normalize, Wo projection.

All big matmuls run at 1 cycle/column: f32r for QK/projections (fp32 data,
TF32-like internal precision), fp16 for PV. K=64 zero-padding keeps the PE
HAM activity monitor warm (2.4 GHz); K=16 matmuls read as ~13% array
activity and leave the PE throttled at 1.2 GHz.

Shapes (hardcoded): vec [8, 1024, 128]; Wq/Wk/Wv [128, 8, 16]; Wo [8, 16, 128].
"""

import sys

sys.path.insert(0, "/opt/trn_rl_repo")

from contextlib import ExitStack

import numpy as np

import concourse.bacc as bacc
import concourse.tile as tile
from concourse import mybir
from concourse.bass_utils import run_bass_kernel_spmd
from concourse.masks import make_identity

F32 = mybir.dt.float32
F32R = mybir.dt.float32r
F16 = mybir.dt.float16
BF16 = mybir.dt.bfloat16
Exp = mybir.ActivationFunctionType.Exp

B, N, X, H, D = 8, 1024, 128, 8, 16
NCHUNK = N // 128          # 8 chunks of 128 along the token dim
SCALE = 0.125              # 1/sqrt(16), halved: S' = 2s - 2m
NR = 4                     # qk rounds: 2 heads each at strips {0, 64}

_CACHED_NC = None


def build_nc():
    """Build the per-core Bass program (identical on all cores)."""
    nc = bacc.Bacc("TRN2")

    # DRAM I/O. Weight tensors arrive pre-permuted from numpy (see kernel()).
    d_wq = [nc.dram_tensor(f"wq{r}", (X, 128), F32, kind="ExternalInput")
            for r in range(NR)]
    d_wk = [nc.dram_tensor(f"wk{r}", (X, 128), F32, kind="ExternalInput")
            for r in range(NR)]
    d_wv = nc.dram_tensor("wv", (X, 128), F32, kind="ExternalInput")
    d_wo = nc.dram_tensor("wo", (128, X), F32, kind="ExternalInput")
    d_vec = nc.dram_tensor("vec", (N, X), F32, kind="ExternalInput")
    d_e8 = nc.dram_tensor("e8c", (H, 128), F32, kind="ExternalInput")
    d_ones = nc.dram_tensor("ones", (1, N), F32, kind="ExternalInput")
    d_out = nc.dram_tensor("out", (N, X), F32, kind="ExternalOutput")

    with tile.TileContext(nc) as tc, ExitStack() as top:
        const = top.enter_context(tc.tile_pool(name="const", bufs=1))
        ident = const.tile([128, 128], F32)
        make_identity(nc, ident)

        # Projections run in full fp32 (accuracy: the QK rounding error of
        # f32r is amplified through exp). QT/KT/ones/m8 are F32R *tiles*
        # (full fp32 bits in SBUF; the TF32-style rounding happens at the PE
        # input) so the S passes stream at 1 cycle/col. f32r tiles must be
        # filled by engine casts or same-dtype DMAs; a bitcast-DMA fill
        # crashes the compiler once a matmul consumes them.
        w_sb = {}
        for name, dram in ([(f"wq{r}", d_wq[r]) for r in range(NR)]
                           + [(f"wk{r}", d_wk[r]) for r in range(NR)]
                           + [("wv", d_wv), ("wo", d_wo)]):
            t = const.tile([128, 128], F32, tag=f"w_{name}", name=f"w_{name}")
            nc.sync.dma_start(out=t[:], in_=dram[:, :])
            w_sb[name] = t
        wo_sb = w_sb["wo"]
        ones_f = const.tile([1, N], F32, tag="ones_f")
        nc.sync.dma_start(out=ones_f[:], in_=d_ones[:, :])
        ones_r = const.tile([1, N], F32R, tag="ones_r")
        nc.scalar.copy(ones_r[:], ones_f[:])   # 0.5-valued (see QT/KT note)

        vecT = const.tile([128, N], F32, tag="vecT")       # [x, n]
        # QT/KT per head, double-packed to K=128: rows [0,16) and [64,80)
        # both hold the head's 16 dims (so every S matmul drives the full
        # 128-partition array -- K=64 f32r reads as 50% HAM activity and
        # the PE gets clock-throttled to 1.2 GHz). Row 16 and 80 are aug
        # rows: KT holds 0.5 in both (so S' = 2s + aug once), QT holds
        # -2*rowmax after the m-dance; remaining rows are zero. The exp
        # scale halves to 0.125 to undo the doubling.
        QT = {h: const.tile([128, N], F32R, tag=f"qt{h}", name=f"qt{h}")
              for h in range(H)}
        KT = {h: const.tile([128, N], F32R, tag=f"kt{h}", name=f"kt{h}")
              for h in range(H)}
        # V layout: [128 j-in-chunk, jc, 17*h + d], col 17h+16 = ones.
        V_sb = const.tile([128, NCHUNK, 17 * H], F16, tag="vsb")
        pt_pool = top.enter_context(tc.tile_pool(name="pt", bufs=4))
        raw_pool = top.enter_context(tc.tile_pool(name="raw", bufs=2))
        recvT = const.tile([128, N], F32, tag="recvT")     # [(h d), i]
        recvN = const.tile([128, N], F32, tag="recvN")     # normalized
        den_sb = const.tile([H, N], F32, tag="den")
        rden = const.tile([H, N], F32, tag="rden")
        e8 = const.tile([H, 128], F32, tag="e8")           # expand matrix
        mha_sb = const.tile([128, NCHUNK, X], F32, tag="mha")

        nc.sync.dma_start(out=e8[:], in_=d_e8[:, :])
        v_heads = V_sb[:].rearrange("p c (h s) -> p c h s", h=H)
        nc.vector.memset(v_heads[:, :, :, 16:17], 1.0)

        # ---- Phase 0: vecT via PE transposes; projections. ----
        with tc.tile_pool(name="stage", bufs=3) as stage, \
                tc.tile_pool(name="ps0", bufs=2, space="PSUM") as ps0, \
                tc.tile_pool(name="ps0b", bufs=2, space="PSUM") as ps0b:
            for c in range(NCHUNK):
                vt = stage.tile([128, 128], F32, tag="vstage")
                nc.sync.dma_start(out=vt[:], in_=d_vec[c * 128:(c + 1) * 128, :])
                pt_ = ps0b.tile([128, 128], F32, tag="trp")
                nc.tensor.transpose(pt_[:, :], vt[:], ident[:])
                nc.scalar.copy(vecT[:, c * 128:(c + 1) * 128], pt_[:, :])

            # QT/KT projections: psum = W.T @ vecT  -> [hd-pos, n];
            # strip t=h%2 of the pair lands at partitions [64t, 64t+17),
            # the rest of that 64-row half is zero (strip-packed W). Copy
            # the full half (strip + zeros) and DMA-duplicate it into the
            # other half: the tile is fully initialized with no memsets.
            for rnd in range(NR):
                for wname, dsts in ((f"wq{rnd}", QT), (f"wk{rnd}", KT)):
                    p = ps0.tile([128, N], F32, tag="proj")
                    for half in range(2):
                        sl = slice(half * 512, (half + 1) * 512)
                        nc.tensor.matmul(p[:, sl], w_sb[wname][:],
                                         vecT[:, sl], start=True, stop=True)
                    for t, h in enumerate((2 * rnd, 2 * rnd + 1)):
                        s0, s1 = 64 * t, 64 * (1 - t)
                        nc.scalar.copy(dsts[h][s0:s0 + 64, :],
                                       p[s0:s0 + 64, :])
                        nc.sync.dma_start(out=dsts[h][s1:s1 + 64, :],
                                          in_=dsts[h][s0:s0 + 64, :])
            # 0.5 rows of KT aug partitions (f32r -> f32r DMA)
            for h in range(H):
                for t in range(2):
                    nc.sync.dma_start(
                        out=KT[h][64 * t + 16:64 * t + 17, :],
                        in_=ones_r[:])

            # V projection: per chunk [j, hd] = vecT[:,chunk].T @ Wv
            for c in range(NCHUNK):
                pv = ps0.tile([128, 128], F32, tag="projv")
                nc.tensor.matmul(pv[:, :], vecT[:, c * 128:(c + 1) * 128],
                                 w_sb["wv"][:], start=True, stop=True)
                dst = V_sb[:, c, :].rearrange("p (h s) -> p h s", h=H)
                src = pv[:, :].rearrange("p (h d) -> p h d", h=H)
                nc.scalar.copy(dst[:, :, 0:16], src[:])

        # ---- Main loop over heads. ----
        with tc.tile_pool(name="small", bufs=6) as small, \
                tc.tile_pool(name="psm", bufs=3, space="PSUM") as psm, \
                tc.tile_pool(name="ppv", bufs=1, space="PSUM") as ppv:
            def emit_form1(rnd, c, m_hs):
                """One i-chunk of the f32r max-pass for both heads of rnd."""
                f1s = {}
                for h in (2 * rnd, 2 * rnd + 1):
                    f1 = psm.tile([128, N], F32, tag="big",
                                  name=f"f1_{h}_{c}")
                    f1s[h] = f1
                    for half in range(2):
                        sl = slice(half * 512, (half + 1) * 512)
                        nc.tensor.matmul(
                            f1[:, sl],
                            QT[h][:, c * 128:(c + 1) * 128],
                            KT[h][:, sl], start=True, stop=True)
                for h in (2 * rnd, 2 * rnd + 1):
                    nc.vector.tensor_reduce(
                        m_hs[h][:, c:c + 1], f1s[h][:, :],
                        axis=mybir.AxisListType.X,
                        op=mybir.AluOpType.max, negate=True)

            def new_mhs(rnd):
                return {h: small.tile([128, NCHUNK], F32, tag="mh",
                                      name=f"mh{h}")
                        for h in (2 * rnd, 2 * rnd + 1)}

            # prologue: round 0 max-pass
            m_cur = new_mhs(0)
            for c in range(NCHUNK):
                emit_form1(0, c, m_cur)

            for rnd in range(NR):
                pair = (2 * rnd, 2 * rnd + 1)

                # m-dance per head: -2*rowmax -> both aug rows of QT[h].
                for h in pair:
                    trp = psm.tile([128, N], F32, tag="big",
                                   name=f"trp{h}")
                    nc.tensor.transpose(trp[0:NCHUNK, 0:128],
                                        m_cur[h][:], ident[:])
                    m8 = small.tile([NCHUNK, 128], F32R, tag="m8",
                                    name=f"m8_{h}")
                    nc.scalar.copy(m8[:], trp[0:NCHUNK, 0:128])
                    nc.sync.dma_start(out=QT[h][16:17, :], in_=m8[:])
                    nc.sync.dma_start(out=QT[h][80:81, :], in_=m8[:])

                # S'^T + exp, strip-interleaved across the head pair,
                # with next round's max-pass chunks woven in.
                m_nxt = new_mhs(rnd + 1) if rnd + 1 < NR else None
                PTs = {h: pt_pool.tile([128, NCHUNK * N], F16, tag="pt",
                                       name=f"pt{h}")
                       for h in pair}
                prv = ppv.tile([128, N], F32, tag="pv", name=f"prv{rnd}")
                def emit_pv(jc):
                    # PV chunk jc, both heads and halves, into one PSUM
                    # tile (partition strips 0:17 / 32:49, col halves).
                    # Lagged one jc behind the ST/exp pipeline so exp(jc)
                    # is already done when the PE reaches it (the in-order
                    # PE queue head-of-line blocks on unmet waits), and the
                    # low-activity M=17 streams average with the K=128 S
                    # matmuls inside the HAM throttle window.
                    for h in pair:
                        po = 32 * (h % 2)
                        for half in range(2):
                            nc.tensor.matmul(
                                prv[po:po + 17,
                                    half * 512:(half + 1) * 512],
                                V_sb[:, jc, 17 * h:17 * h + 17],
                                PTs[h][:, jc * N + half * 512:
                                        jc * N + (half + 1) * 512],
                                start=(jc == 0), stop=(jc == NCHUNK - 1))

                for jc in range(NCHUNK):
                    if m_nxt is not None:
                        emit_form1(rnd + 1, jc, m_nxt)
                    sts = {}
                    for h in pair:
                        st = psm.tile([128, N], F32, tag="big",
                                      name=f"st_{h}_{jc}")
                        sts[h] = st
                        for half in range(2):
                            sl = slice(half * 512, (half + 1) * 512)
                            nc.tensor.matmul(
                                st[:, sl],
                                KT[h][:, jc * 128:(jc + 1) * 128],
                                QT[h][:, sl], start=True, stop=True)
                    for h in pair:
                        nc.scalar.activation(
                            PTs[h][:, jc * N:jc * N + N], sts[h][:, :],
                            Exp, bias=0.0, scale=SCALE)
                    if jc > 0:
                        emit_pv(jc - 1)
                emit_pv(NCHUNK - 1)
                rawt = raw_pool.tile([64, N], F32, tag="raw",
                                     name=f"raw{rnd}")
                nc.vector.tensor_copy(rawt[0:49, :], prv[0:49, :])
                for h in pair:
                    po = 32 * (h % 2)
                    nc.sync.dma_start(out=recvT[16 * h:16 * h + 16, :],
                                      in_=rawt[po:po + 16, :])
                    nc.sync.dma_start(out=den_sb[h:h + 1, :],
                                      in_=rawt[po + 16:po + 17, :])
                m_cur = m_nxt

        # ---- Tail: normalize + output projection. ----
        with tc.tile_pool(name="pst", bufs=2, space="PSUM") as pst, \
                tc.tile_pool(name="pstb", bufs=2, space="PSUM") as pstb:
            nc.scalar.activation(rden[:], den_sb[:],
                                 mybir.ActivationFunctionType.Ln,
                                 bias=0.0, scale=1.0)
            nc.scalar.activation(rden[:], rden[:], Exp, bias=0.0, scale=-1.0)
            pe_ = pst.tile([128, N], F32, tag="expand")
            for half in range(2):
                sl = slice(half * 512, (half + 1) * 512)
                nc.tensor.matmul(pe_[:, sl], e8[:], rden[:, sl],
                                 start=True, stop=True)
            nc.vector.tensor_mul(recvN[:], recvT[:], pe_[:, :])
            for c in range(NCHUNK):
                po = pstb.tile([128, 128], F32, tag="mha")
                nc.tensor.matmul(po[:, :], recvN[:, c * 128:(c + 1) * 128],
                                 wo_sb[:], start=True, stop=True)
                nc.scalar.copy(mha_sb[:, c, :], po[:, :])
                nc.sync.dma_start(out=d_out[c * 128:(c + 1) * 128, :],
                                  in_=mha_sb[:, c, :])

    nc.finalize()
    return nc


def _permute_weights(Wq, Wk, Wv, Wo):
    """Numpy-side weight layout prep: strip-pack with K=64 zero padding."""
    def strip_pack(W, heads):
        out = np.zeros((X, 128), dtype=np.float32)
        for t, h in enumerate(heads):
            out[:, 64 * t:64 * t + 16] = W[:, h, :]
        return out

    e8c = np.zeros((H, 128), dtype=np.float32)
    for h in range(H):
        e8c[h, 16 * h:16 * h + 16] = 1.0
    d = dict(
        wv=np.ascontiguousarray(Wv.reshape(X, 128)),
        wo=np.ascontiguousarray(Wo.reshape(128, X)),
        e8c=e8c, ones=np.full((1, N), 0.5, dtype=np.float32),
    )
    for r in range(NR):
        d[f"wq{r}"] = strip_pack(Wq, [2 * r, 2 * r + 1])
        d[f"wk{r}"] = strip_pack(Wk, [2 * r, 2 * r + 1])
    return d


def kernel(Wq, Wk, Wv, Wo, vec, trace=False, tmpdir=None):
    global _CACHED_NC
    if _CACHED_NC is None:
        _CACHED_NC = build_nc()
    nc = _CACHED_NC

    w = _permute_weights(np.asarray(Wq, np.float32), np.asarray(Wk, np.float32),
                         np.asarray(Wv, np.float32), np.asarray(Wo, np.float32))
    vec = np.asarray(vec, np.float32)
    in_maps = [dict(w, vec=np.ascontiguousarray(vec[b])) for b in range(B)]
    res = run_bass_kernel_spmd(nc, in_maps, core_ids=list(range(B)),
                               trace=trace, tmpdir=tmpdir)
    out = np.stack([res.results[b]["out"] for b in range(B)])
    if trace:
        return out, res
    return out
